# revision 10
# baseline (speedup 1.0000x reference)
"""Trainium2 Bass kernel for GQA causal sliding-window self-attention.

Problem: B=2, T=2048, C=1024, 16 heads (hd=64), 4 KV groups, window=256.
  q = x@Wq+bq; k = x@Wk+bk; v = x@Wv+bv  (GQA repeat of kv over 4 heads)
  att = softmax(mask(q k^T / 8));  y = (att v) @ Wo + bo

Sharding: data-parallel over (batch, T-chunk). 8 cores = 2 batches x 4
chunks of 512 query tokens. Each core receives the 768-token extended
x slice (512 queries + 256 halo for the window), computes its chunk's
output rows completely locally; no collectives.

Device layout (per core): everything is computed "transposed" (feature
dim on partitions) so that the TensorE contraction dim is always on
partitions and softmax denominators come out of the att@v matmul for
free via a ones-column appended to V:
  xT [1024c, 768t] -> qT [1024, 512], kT (per-group, duplicated into both
  64-partition halves) [128, 768], v token-major [128t, 4g, 64+1].
  scoresT[kj, qi] blocks of [512kj, 256qi] per (head, q-block);
  exp via ScalarE; att@v gives yT_aug [65, 256] whose row 64 is the
  softmax denominator; reciprocal + K=1 broadcast matmul + DVE multiply
  normalizes; final projection uses yT as lhsT so the output psum is
  token-major, DMA'd straight out.

All matmuls use float32r (full fp32 inputs, 1 cycle/row at N>=256).

Host folds 1/sqrt(64) into Wq/bq, and applies the exactly-linear bv/bo
corrections after the device pass:  out += bv_rep @ Wo + bo.
"""

import sys

sys.path.insert(0, "/opt/trn_rl_repo")

import numpy as np

import concourse.bass as bass
import concourse.tile as tile
from concourse import mybir
from concourse.bass_utils import run_bass_kernel_spmd
from concourse.vector_clock import ScopedClock

F32 = mybir.dt.float32
F32R = mybir.dt.float32r

B, T, C = 2, 2048, 1024
NH, NG, HD = 16, 4, 64
KV = NG * HD  # 256
WINDOW = 256
NCORES = 8
TQ = 512  # query tokens per core
TE = TQ + WINDOW  # 768 extended tokens per core
NEG = -1e9


class _ChunkedDrainTileContext(tile.TileContext):
    """Walrus in this container only accepts 1 sync wait on CTRL-class
    instructions; the stock Tile tail drain carries one wait per
    outstanding proc. Spread them over SP nops first."""

    def _drain_and_barrier(self, tick_clock, wait_clock):
        gc = tick_clock.global_clock
        entries = []
        for scope, vc in ScopedClock({None: gc}).items():
            for proc in range(len(vc)):
                t = vc[proc]
                if t > 0:
                    entries.append((scope, proc, t))
        cur = ScopedClock()
        for scope, proc, t in entries:
            nop = self.nc.sync.nop(nofuse=True, hint="tail_wait")
            partial = ScopedClock()
            partial.require_at_least(scope, proc, t)
            wait_clock.add_sem_waits(nop.ins, partial, cur)
            cur.update_past(partial)
        drain_inst = self.nc.sync.drain()
        wait_clock.add_sem_waits(drain_inst.ins, ScopedClock({None: gc}), cur)
        self.nc.all_engine_barrier()
        assert self.sems is not None
        popped = self.nc._tile_sem_poison_stack.pop()
        assert popped is self._sem_poison
        self.nc.clear_and_free_semaphores(list(self.sems.allocated().values()))
        self.nc.all_engine_barrier()


def _r(ap):
    return ap.bitcast(F32R)


def _split_multi_waits(nc, max_waits=1):
    """This walrus build rejects >1 sync wait on several instruction structs
    (CTRL, self-loading fp32r Matmult). Hoist excess waits onto same-engine
    NOPs placed immediately before the instruction — identical semantics."""
    fn = nc.m.functions[0]
    for blk in fn.blocks:
        insts = blk.instructions
        new = []
        changed = False
        for inst in insts:
            si = inst.sync_info
            waits = list(si.on_wait) if si is not None and si.on_wait else []
            if len(waits) > max_waits:
                changed = True
                for w in waits[:-max_waits]:
                    nop = mybir.InstNoOp(
                        name=nc.get_next_instruction_name(),
                        ins=[],
                        outs=[],
                        engine=inst.engine,
                        sync_info=mybir.SyncInfo(on_wait=[w], on_update=[]),
                        bass_nofuse=True,
                    )
                    new.append(nop)
                si.on_wait = waits[-max_waits:]
                inst.sync_info = si
            new.append(inst)
        if changed:
            blk.instructions = new


def _build_program():
    nc = bass.Bass("TRN2", target_bir_lowering=False, debug=False, num_devices=NCORES)

    xt = nc.dram_tensor("xt", [C, TE], F32, kind="ExternalInput")
    wq = nc.dram_tensor("wq", [C, C], F32, kind="ExternalInput")
    wk = nc.dram_tensor("wk", [C, KV], F32, kind="ExternalInput")
    wv = nc.dram_tensor("wv", [C, KV], F32, kind="ExternalInput")
    wo = nc.dram_tensor("wo", [C, C], F32, kind="ExternalInput")
    bq = nc.dram_tensor("bq", [C, 1], F32, kind="ExternalInput")
    bk = nc.dram_tensor("bk", [KV, 1], F32, kind="ExternalInput")
    maskp = nc.dram_tensor("maskp", [128, 4, TQ], F32, kind="ExternalInput")
    out = nc.dram_tensor("out", [TQ, C], F32, kind="ExternalOutput")

    KC = C // 128  # 8 contraction tiles

    with _ChunkedDrainTileContext(nc) as tc:
        import contextlib

        with contextlib.ExitStack() as ctx:
            wsb = ctx.enter_context(tc.tile_pool(name="wsb", bufs=1))
            xsb = ctx.enter_context(tc.tile_pool(name="xsb", bufs=1))
            csb = ctx.enter_context(tc.tile_pool(name="csb", bufs=1))
            qkv = ctx.enter_context(tc.tile_pool(name="qkv", bufs=1))
            ynp = ctx.enter_context(tc.tile_pool(name="ynp", bufs=1))
            expp = ctx.enter_context(tc.tile_pool(name="expp", bufs=3))
            rrp = ctx.enter_context(tc.tile_pool(name="rrp", bufs=4))
            outp = ctx.enter_context(tc.tile_pool(name="outp", bufs=2))
            ssp = ctx.enter_context(tc.tile_pool(name="ssp", bufs=3))
            pj = ctx.enter_context(tc.tile_pool(name="pj", bufs=2, space="PSUM"))
            scp_pool = ctx.enter_context(tc.tile_pool(name="scp", bufs=2, space="PSUM"))
            ytp_pool = ctx.enter_context(tc.tile_pool(name="ytp", bufs=2, space="PSUM"))

            # ---- loads ----
            xt_sb = []
            wk_sb = []
            wv_sb = []
            wq_sb = []
            wo_sb = []
            for kc in range(KC):
                t = xsb.tile([128, TE], F32, name=f"xt{kc}", tag=f"xt{kc}")
                nc.gpsimd.dma_start(out=_r(t[:]), in_=_r(xt[kc * 128 : (kc + 1) * 128, :]))
                xt_sb.append(t)
                t = wsb.tile([128, KV], F32, name=f"wk{kc}", tag=f"wk{kc}")
                nc.gpsimd.dma_start(out=_r(t[:]), in_=_r(wk[kc * 128 : (kc + 1) * 128, :]))
                wk_sb.append(t)
                t = wsb.tile([128, KV], F32, name=f"wv{kc}", tag=f"wv{kc}")
                nc.gpsimd.dma_start(out=_r(t[:]), in_=_r(wv[kc * 128 : (kc + 1) * 128, :]))
                wv_sb.append(t)
            for kc in range(KC):
                t = wsb.tile([128, C], F32, name=f"wq{kc}", tag=f"wq{kc}")
                nc.gpsimd.dma_start(out=_r(t[:]), in_=_r(wq[kc * 128 : (kc + 1) * 128, :]))
                wq_sb.append(t)
            mask_sb = csb.tile([128, 4, TQ], F32)
            nc.gpsimd.dma_start(out=mask_sb[:], in_=maskp[:])
            bq_sb = []
            for m in range(KC):
                t = csb.tile([128, 1], F32, name=f"bq{m}", tag=f"bq{m}")
                nc.gpsimd.dma_start(out=t[:], in_=bq[m * 128 : (m + 1) * 128, :])
                bq_sb.append(t)
            bk_sb = []
            for m in range(2):
                t = csb.tile([128, 1], F32, name=f"bk{m}", tag=f"bk{m}")
                nc.gpsimd.dma_start(out=t[:], in_=bk[m * 128 : (m + 1) * 128, :])
                bk_sb.append(t)
            for kc in range(KC):
                t = wsb.tile([128, C], F32, name=f"wo{kc}", tag=f"wo{kc}")
                nc.gpsimd.dma_start(out=_r(t[:]), in_=_r(wo[kc * 128 : (kc + 1) * 128, :]))
                wo_sb.append(t)
            ones64_f = csb.tile([1, 64], F32)
            nc.vector.memset(ones64_f[:], 1.0)
            ones64 = csb.tile([1, 64], F32)
            nc.vector.tensor_copy(_r(ones64[:]), ones64_f[:])
            onescol_f = csb.tile([128, NG, 1], F32)
            nc.vector.memset(onescol_f[:], 1.0)

            # ---- kT projection: kT[g][dup_half*64+d, te] ----
            kT_sb = [qkv.tile([128, TE], F32, name=f"kT{g}", tag=f"kT{g}") for g in range(NG)]
            for mt in range(2):  # kv partition tile (2 groups each)
                for s2 in range(2):  # token span halves of 384
                    kp = pj.tile([128, 512], F32, name="kp", tag="pj")
                    for kc in range(KC):
                        nc.tensor.matmul(
                            kp[:, 0:384],
                            _r(wk_sb[kc][:, mt * 128 : (mt + 1) * 128]),
                            _r(xt_sb[kc][:, s2 * 384 : (s2 + 1) * 384]),
                            start=(kc == 0),
                            stop=(kc == KC - 1),
                        )
                    for gh in range(2):  # source half (group g = 2*mt+gh)
                        g = 2 * mt + gh
                        for half in range(2):  # dest duplicated half
                            nc.vector.tensor_scalar_add(
                                _r(kT_sb[g][half * 64 : half * 64 + 64, s2 * 384 : (s2 + 1) * 384]),
                                kp[gh * 64 : gh * 64 + 64, 0:384],
                                bk_sb[mt][gh * 64 : gh * 64 + 64, :],
                            )

            # ---- v projection: token-major with ones column ----
            v_sb = [qkv.tile([128, NG, HD + 1], F32, name=f"v{vt}", tag=f"v{vt}") for vt in range(6)]
            for vt in range(6):
                vp = pj.tile([128, 512], F32, name="vp", tag="pj")
                for kc in range(KC):
                    nc.tensor.matmul(
                        vp[:, 0:KV],
                        _r(xt_sb[kc][:, vt * 128 : (vt + 1) * 128]),
                        _r(wv_sb[kc][:]),
                        start=(kc == 0),
                        stop=(kc == KC - 1),
                    )
                nc.vector.tensor_copy(
                    _r(v_sb[vt][:, :, 0:HD]),
                    vp[:, 0:KV].rearrange("p (g d) -> p g d", g=NG),
                )
                nc.vector.tensor_copy(_r(v_sb[vt][:, :, HD : HD + 1]), onescol_f[:])

            # ---- qT projection ----
            qT_sb = [qkv.tile([128, TQ], F32, name=f"qT{m}", tag=f"qT{m}") for m in range(KC)]
            for m in range(KC):
                qp = pj.tile([128, 512], F32, name="qp", tag="pj")
                for kc in range(KC):
                    nc.tensor.matmul(
                        qp[:],
                        _r(wq_sb[kc][:, m * 128 : (m + 1) * 128]),
                        _r(xt_sb[kc][:, WINDOW:TE]),
                        start=(kc == 0),
                        stop=(kc == KC - 1),
                    )
                nc.vector.tensor_scalar_add(_r(qT_sb[m][:]), qp[:], bq_sb[m][:])

            # ---- attention ----
            yn_sb = [ynp.tile([128, TQ], F32, name=f"yn{m}", tag=f"yn{m}") for m in range(KC)]
            for qb in range(2):
                for h in range(NH):
                    m, hh, g = h // 2, h % 2, h // 4
                    scp = scp_pool.tile([128, 4, 256], F32, name="scp", tag="sc")
                    for kt in range(4):
                        ke0 = qb * 256 + kt * 128
                        nc.tensor.matmul(
                            scp[:, kt, :],
                            _r(kT_sb[g][hh * 64 : hh * 64 + 64, ke0 : ke0 + 128]),
                            _r(qT_sb[m][hh * 64 : hh * 64 + 64, qb * 256 : qb * 256 + 256]),
                            start=True,
                            stop=True,
                        )
                    nc.vector.tensor_tensor(
                        scp[:],
                        scp[:],
                        mask_sb[:, :, qb * 256 : qb * 256 + 256],
                        mybir.AluOpType.add,
                    )
                    ex = expp.tile([128, 4, 256], F32, name="ex", tag="ex")
                    nc.scalar.activation(_r(ex[:]), scp[:], mybir.ActivationFunctionType.Exp)
                    ytp = ytp_pool.tile([HD + 1, 256], F32, name="ytp", tag="yt")
                    for kt in range(4):
                        vt = qb * 2 + kt
                        nc.tensor.matmul(
                            ytp[:],
                            _r(v_sb[vt][:, g, :]),
                            _r(ex[:, kt, :]),
                            start=(kt == 0),
                            stop=(kt == 3),
                        )
                    rr = rrp.tile([1, 256], F32, name="rr", tag="rr")
                    with nc.allow_low_precision(reason="fp32r rounding of softmax denom reciprocal"):
                        nc.vector.reciprocal(_r(rr[:]), ytp[HD : HD + 1, :])
                    sp = pj.tile([128, 512], F32, name="sp", tag="pj")
                    nc.tensor.matmul(
                        sp[0:64, 0:256], _r(ones64[:]), _r(rr[:]), start=True, stop=True
                    )
                    ssb = ssp.tile([64, 256], F32, name="ssb", tag="ssb")
                    nc.scalar.copy(ssb[:], sp[0:64, 0:256])
                    nc.vector.tensor_tensor(
                        _r(yn_sb[m][hh * 64 : hh * 64 + 64, qb * 256 : qb * 256 + 256]),
                        ytp[0:HD, :],
                        ssb[:],
                        mybir.AluOpType.mult,
                    )

            # ---- output projection (token-major out) ----
            for tt in range(4):
                ob = outp.tile([128, C], F32, name="ob", tag="ob")
                for n2 in range(2):
                    op = pj.tile([128, 512], F32, name="op", tag="pj")
                    for kc in range(KC):
                        nc.tensor.matmul(
                            op[:],
                            _r(yn_sb[kc][:, tt * 128 : (tt + 1) * 128]),
                            _r(wo_sb[kc][:, n2 * 512 : (n2 + 1) * 512]),
                            start=(kc == 0),
                            stop=(kc == KC - 1),
                        )
                    nc.scalar.copy(ob[:, n2 * 512 : (n2 + 1) * 512], op[:])
                nc.gpsimd.dma_start(
                    out=out[tt * 128 : (tt + 1) * 128, :], in_=ob[:]
                )

    _split_multi_waits(nc)
    return nc


_NC = None


def _get_nc():
    global _NC
    if _NC is None:
        _NC = _build_program()
    return _NC


def _host_prep(x, Wq, bq, Wk, bk, Wv, bv, Wo, bo):
    x = np.ascontiguousarray(np.asarray(x, dtype=np.float32))
    Wq = np.asarray(Wq, np.float32)
    bq = np.asarray(bq, np.float32)
    Wk = np.asarray(Wk, np.float32)
    bk = np.asarray(bk, np.float32)
    Wv = np.asarray(Wv, np.float32)
    bv = np.asarray(bv, np.float32)
    Wo = np.asarray(Wo, np.float32)
    bo = np.asarray(bo, np.float32)

    scale = np.float32(1.0 / np.sqrt(HD))
    wq_h = np.ascontiguousarray(Wq * scale)
    bq_h = np.ascontiguousarray((bq * scale).reshape(C, 1))
    bk_h = np.ascontiguousarray(bk.reshape(KV, 1))

    # band masks, block-packed: maskp[p, kt, qb*256+qi] for kj_l = kt*128+p
    kj = np.arange(2 * WINDOW)[:, None]  # 512 local k indices within a q-block
    qi = np.arange(WINDOW)[None, :]  # 256 local q indices within a q-block
    band = (qi <= kj) & (kj <= qi + WINDOW)  # same for every block
    masks = {}
    for c in range(4):
        mk = np.empty((128, 4, TQ), np.float32)
        for qb in range(2):
            valid = band.copy()
            if c == 0 and qb == 0:
                valid &= kj >= WINDOW  # global j >= 0 at the sequence start
            mkb = np.where(valid, np.float32(0.0), np.float32(NEG))
            mk[:, :, qb * 256 : (qb + 1) * 256] = (
                mkb.reshape(4, 128, WINDOW).transpose(1, 0, 2)
            )
        masks[c] = mk

    in_maps = []
    for core in range(NCORES):
        b, c = core // 4, core % 4
        t0 = c * TQ - WINDOW
        xe = np.zeros((TE, C), np.float32)
        lo = max(t0, 0)
        xe[lo - t0 : TE, :] = x[b, lo : t0 + TE, :]
        in_maps.append(
            {
                "xt": np.ascontiguousarray(xe.T),
                "wq": wq_h,
                "wk": np.ascontiguousarray(Wk),
                "wv": np.ascontiguousarray(Wv),
                "wo": np.ascontiguousarray(Wo),
                "bq": bq_h,
                "bk": bk_h,
                "maskp": masks[c],
            }
        )

    # exact linear bias correction applied host-side:
    # y = att@(v+bv) = att@v + bv (softmax rows sum to 1), so
    # out += bv_rep @ Wo + bo
    bv_rep = np.concatenate([bv[(h // NG) * HD : (h // NG + 1) * HD] for h in range(NH)])
    corr = bv_rep.astype(np.float64) @ Wo.astype(np.float64) + bo.astype(np.float64)
    return in_maps, corr.astype(np.float32)


LAST_RESULTS = None


def kernel(x, Wq, bq, Wk, bk, Wv, bv, Wo, bo):
    global LAST_RESULTS
    in_maps, corr = _host_prep(x, Wq, bq, Wk, bk, Wv, bv, Wo, bo)
    nc = _get_nc()
    res = run_bass_kernel_spmd(nc, in_maps, core_ids=list(range(NCORES)))
    LAST_RESULTS = res
    out = np.empty((B, T, C), np.float32)
    for core in range(NCORES):
        b, c = core // 4, core % 4
        out[b, c * TQ : (c + 1) * TQ, :] = res.results[core]["out"]
    out += corr[None, None, :]
    return out


# revision 28
# speedup vs baseline: 629.5497x; 629.5497x over previous
"""Trainium2 Bass kernel for GQA causal sliding-window self-attention.

Problem: B=2, T=2048, C=1024, 16 heads (hd=64), 4 KV groups, window=256.
  q = x@Wq+bq; k = x@Wk+bk; v = x@Wv+bv  (GQA repeat of kv over 4 heads)
  att = softmax(mask(q k^T / 8));  y = (att v) @ Wo + bo

Sharding: data-parallel over (batch, T-chunk). 8 cores = 2 batches x 4
chunks of 512 query tokens. Each core receives the 768-token extended
x slice (512 queries + 256 halo for the window) and computes its chunk's
output rows completely locally; no collectives.

Device layout (per core): everything is computed "transposed" (feature
dim on partitions) so the TensorE contraction dim is always on
partitions and softmax denominators come out of the att@v matmul for
free via a ones-column appended to V:
  xT [1024c, 768t] -> qT [1024, 512], kT (per-group, duplicated into both
  64-partition halves) [128, 768], v token-major [128t, 4g, 64+1].
  scoresT[kj, qi] blocks of [512kj, 256qi] per (head, q-block);
  mask-add on DVE, exp on ScalarE; att@v gives yT_aug [65, 256] whose
  row 64 is the softmax denominator; reciprocal + K=1 broadcast matmul +
  DVE multiply normalizes; the final projection uses yT as lhsT so the
  output psum is token-major and is DMA'd straight out.

All matmuls use float32r (fp32 storage, 1 cycle/row at N>=256).

Wq is passed (m, kc)-tiled so the q projection for the first heads can
start as soon as its first 0.5 MB arrives instead of after the full
4 MB. The output projection for a q-block is emitted right after that
block's heads so PE has fill-in work while attention drains.

Host folds 1/sqrt(64) into Wq/bq and applies the exactly-linear bv/bo
corrections after the device pass:  out += bv_rep @ Wo + bo.
"""

import sys

sys.path.insert(0, "/opt/trn_rl_repo")

import contextlib

import numpy as np

import concourse.bass as bass
import concourse.tile as tile
from concourse import mybir
from concourse.bass_utils import run_bass_kernel_spmd
from concourse import library_config
from concourse.vector_clock import ScopedClock

F32 = mybir.dt.float32
F32R = mybir.dt.float32r

B, T, C = 2, 2048, 1024
NH, NG, HD = 16, 4, 64
KV = NG * HD  # 256
WINDOW = 256
NCORES = 8
TQ = 512  # query tokens per core
TE = TQ + WINDOW  # 768 extended tokens per core
NEG = -1e9
KC = C // 128  # 8 contraction tiles


class _ChunkedDrainTileContext(tile.TileContext):
    """Walrus in this container only accepts 1 sync wait on CTRL-class
    instructions; the stock Tile tail drain carries one wait per
    outstanding proc. Spread them over SP nops first, and use the cheaper
    sem-only barriers for the tail."""

    def _drain_and_barrier(self, tick_clock, wait_clock):
        gc = tick_clock.global_clock
        entries = []
        for scope, vc in ScopedClock({None: gc}).items():
            for proc in range(len(vc)):
                t = vc[proc]
                if t > 0:
                    entries.append((scope, proc, t))
        cur = ScopedClock()
        for scope, proc, t in entries:
            nop = self.nc.sync.nop(nofuse=True, hint="tail_wait")
            partial = ScopedClock()
            partial.require_at_least(scope, proc, t)
            wait_clock.add_sem_waits(nop.ins, partial, cur)
            cur.update_past(partial)
        drain_inst = self.nc.sync.drain()
        wait_clock.add_sem_waits(drain_inst.ins, ScopedClock({None: gc}), cur)
        self.nc.all_engine_barrier(sem_only=True)
        assert self.sems is not None
        popped = self.nc._tile_sem_poison_stack.pop()
        assert popped is self._sem_poison
        self.nc.clear_and_free_semaphores(list(self.sems.allocated().values()))
        self.nc.all_engine_barrier(sem_only=True)


def _r(ap):
    return ap.bitcast(F32R)


def _split_multi_waits(nc, max_waits=1):
    """This walrus build rejects >1 sync wait on several instruction structs
    (CTRL, self-loading fp32r Matmult). Hoist excess waits onto same-engine
    NOPs placed immediately before the instruction — identical semantics."""
    fn = nc.m.functions[0]
    for blk in fn.blocks:
        insts = blk.instructions
        new = []
        changed = False
        for inst in insts:
            si = inst.sync_info
            waits = list(si.on_wait) if si is not None and si.on_wait else []
            if len(waits) > max_waits:
                changed = True
                for w in waits[:-max_waits]:
                    nop = mybir.InstNoOp(
                        name=nc.get_next_instruction_name(),
                        ins=[],
                        outs=[],
                        engine=inst.engine,
                        sync_info=mybir.SyncInfo(on_wait=[w], on_update=[]),
                        bass_nofuse=True,
                    )
                    nc.register_instruction(nop, overwrite=True)
                    new.append(nop)
                si.on_wait = waits[-max_waits:]
                inst.sync_info = si
            new.append(inst)
        if changed:
            blk.instructions = new


def _build_program():
    nc = bass.Bass("TRN2", target_bir_lowering=False, debug=False, num_devices=NCORES)

    xt = nc.dram_tensor("xt", [128, KC, TE], F32, kind="ExternalInput")
    wq = nc.dram_tensor("wq", [KC, 128, KC, 128], F32, kind="ExternalInput")  # [m][p][kc]
    wk = nc.dram_tensor("wk", [128, KC, KV], F32, kind="ExternalInput")
    wv = nc.dram_tensor("wv", [128, KC, KV], F32, kind="ExternalInput")
    wo = nc.dram_tensor("wo", [128, KC, C], F32, kind="ExternalInput")
    bq = nc.dram_tensor("bq", [C, 1], F32, kind="ExternalInput")
    bk = nc.dram_tensor("bk", [KV, 1], F32, kind="ExternalInput")
    maskp = nc.dram_tensor("maskp", [128, 4, TQ], F32, kind="ExternalInput")
    out = nc.dram_tensor("out", [TQ, C], F32, kind="ExternalOutput")

    with _ChunkedDrainTileContext(nc) as tc:
        with contextlib.ExitStack() as ctx:
            wsb = ctx.enter_context(tc.tile_pool(name="wsb", bufs=1))
            xsb = ctx.enter_context(tc.tile_pool(name="xsb", bufs=1))
            csb = ctx.enter_context(tc.tile_pool(name="csb", bufs=1))
            qkv = ctx.enter_context(tc.tile_pool(name="qkv", bufs=1))
            ynp = ctx.enter_context(tc.tile_pool(name="ynp", bufs=1))
            expp = ctx.enter_context(tc.tile_pool(name="expp", bufs=3))
            rrp = ctx.enter_context(tc.tile_pool(name="rrp", bufs=4))
            outp = ctx.enter_context(tc.tile_pool(name="outp", bufs=2))
            ytsp = ctx.enter_context(tc.tile_pool(name="ytsp", bufs=3))
            rbp = ctx.enter_context(tc.tile_pool(name="rbp", bufs=3))
            pj = ctx.enter_context(tc.tile_pool(name="pj", bufs=3, space="PSUM"))
            scp_pool = ctx.enter_context(tc.tile_pool(name="scp", bufs=3, space="PSUM"))
            ytp_pool = ctx.enter_context(tc.tile_pool(name="ytp", bufs=2, space="PSUM"))

            # ---- loads (few big DMAs, ordered by consumption deadline) ----
            wk_all = wsb.tile([128, KC, KV], F32, name="wk_all", tag="wk_all")
            nc.sync.dma_start(out=_r(wk_all[:]), in_=_r(wk[:]))
            wv_all = wsb.tile([128, KC, KV], F32, name="wv_all", tag="wv_all")
            nc.sync.dma_start(out=_r(wv_all[:]), in_=_r(wv[:]))
            xt_all = xsb.tile([128, KC, TE], F32, name="xt_all", tag="xt_all")
            nc.sync.dma_start(out=_r(xt_all[:]), in_=_r(xt[:]))
            bq_all = csb.tile([128, KC], F32)
            nc.sync.dma_start(out=bq_all[:], in_=bq[:, 0].rearrange("(m p) -> p m", p=128))
            bk_all = csb.tile([128, 2], F32)
            nc.sync.dma_start(out=bk_all[:], in_=bk[:, 0].rearrange("(m p) -> p m", p=128))
            ones64_f = csb.tile([1, 64], F32)
            nc.vector.memset(ones64_f[:], 1.0)
            ones64 = csb.tile([1, 64], F32)
            nc.vector.tensor_copy(_r(ones64[:]), ones64_f[:])
            onescol_f = csb.tile([128, NG, 1], F32)
            nc.vector.memset(onescol_f[:], 1.0)

            # Wq arrives (m, kc)-tiled so head-pair m can start after 0.5 MB.
            wq_sb = [None] * KC

            def _load_wq(m):
                t = wsb.tile([128, KC, 128], F32, name=f"wq{m}", tag=f"wq{m}")
                nc.sync.dma_start(out=_r(t[:]), in_=_r(wq[m, :, :, :]))
                wq_sb[m] = t

            for m in range(2):
                _load_wq(m)
            mask_sb = csb.tile([128, 4, TQ], F32)
            nc.sync.dma_start(out=mask_sb[:], in_=maskp[:])
            for m in range(2, KC):
                _load_wq(m)
            wo_all = wsb.tile([128, KC, C], F32, name="wo_all", tag="wo_all")
            nc.sync.dma_start(out=_r(wo_all[:]), in_=_r(wo[:]))

            # ---- kT projection: kT[g][dup_half*64+d, te] ----
            kT_sb = [qkv.tile([128, TE], F32, name=f"kT{g}", tag=f"kT{g}") for g in range(NG)]
            for mt in range(2):  # kv partition tile (2 groups each)
                for s2 in range(2):  # token span halves of 384
                    kp = pj.tile([128, 512], F32, name="kp", tag="pj")
                    for kc in range(KC):
                        nc.tensor.matmul(
                            kp[:, 0:384],
                            _r(wk_all[:, kc, mt * 128 : (mt + 1) * 128]),
                            _r(xt_all[:, kc, s2 * 384 : (s2 + 1) * 384]),
                            start=(kc == 0),
                            stop=(kc == KC - 1),
                        )
                    for gh in range(2):  # source half (group g = 2*mt+gh)
                        g = 2 * mt + gh
                        for half in range(2):  # dest duplicated half
                            nc.vector.tensor_scalar_add(
                                _r(kT_sb[g][half * 64 : half * 64 + 64, s2 * 384 : (s2 + 1) * 384]),
                                kp[gh * 64 : gh * 64 + 64, 0:384],
                                bk_all[gh * 64 : gh * 64 + 64, mt : mt + 1],
                            )

            # ---- v projection: token-major with ones column ----
            v_sb = [qkv.tile([128, NG, HD + 1], F32, name=f"v{vt}", tag=f"v{vt}") for vt in range(6)]
            for vt in range(6):
                vp = pj.tile([128, 512], F32, name="vp", tag="pj")
                for kc in range(KC):
                    nc.tensor.matmul(
                        vp[:, 0:KV],
                        _r(xt_all[:, kc, vt * 128 : (vt + 1) * 128]),
                        _r(wv_all[:, kc, :]),
                        start=(kc == 0),
                        stop=(kc == KC - 1),
                    )
                nc.scalar.copy(
                    _r(v_sb[vt][:, :, 0:HD]),
                    vp[:, 0:KV].rearrange("p (g d) -> p g d", g=NG),
                )
                nc.scalar.copy(_r(v_sb[vt][:, :, HD : HD + 1]), onescol_f[:])

            yn_sb = [ynp.tile([128, TQ], F32, name=f"yn{m}", tag=f"yn{m}") for m in range(KC)]
            for m in range(KC):
                # ---- qT projection for head pair m ----
                qp = pj.tile([128, 512], F32, name="qp", tag="pj")
                for kc in range(KC):
                    nc.tensor.matmul(
                        qp[:],
                        _r(wq_sb[m][:, kc, :]),
                        _r(xt_all[:, kc, WINDOW:TE]),
                        start=(kc == 0),
                        stop=(kc == KC - 1),
                    )
                qT = qkv.tile([128, TQ], F32, name=f"qT{m}", tag=f"qT{m}")
                nc.vector.tensor_scalar_add(_r(qT[:]), qp[:], bq_all[:, m : m + 1])

                # ---- attention for heads 2m, 2m+1 ----
                for h in (2 * m, 2 * m + 1):
                    hh, g = h % 2, h // 4
                    for qb in range(2):
                        scpA = scp_pool.tile([128, 2, 256], F32, name="scpA", tag="sc")
                        scpB = scp_pool.tile([128, 2, 256], F32, name="scpB", tag="sc")
                        halves = (scpA, scpB)
                        for kt in range(4):
                            ke0 = qb * 256 + kt * 128
                            nc.tensor.matmul(
                                halves[kt // 2][:, kt % 2, :],
                                _r(kT_sb[g][hh * 64 : hh * 64 + 64, ke0 : ke0 + 128]),
                                _r(qT[hh * 64 : hh * 64 + 64, qb * 256 : qb * 256 + 256]),
                                start=True,
                                stop=True,
                            )
                        ex = expp.tile([128, 4, 256], F32, name="ex", tag="ex")
                        for half in range(2):
                            nc.vector.tensor_tensor(
                                halves[half][:],
                                halves[half][:],
                                mask_sb[:, 2 * half : 2 * half + 2, qb * 256 : qb * 256 + 256],
                                mybir.AluOpType.add,
                            )
                            nc.scalar.activation(
                                _r(ex[:, 2 * half : 2 * half + 2, :]),
                                halves[half][:],
                                mybir.ActivationFunctionType.Exp,
                            )
                        ytp = ytp_pool.tile([HD + 1, 256], F32, name="ytp", tag="yt")
                        for kt in range(4):
                            vt = qb * 2 + kt
                            nc.tensor.matmul(
                                ytp[:],
                                _r(v_sb[vt][:, g, :]),
                                _r(ex[:, kt, :]),
                                start=(kt == 0),
                                stop=(kt == 3),
                            )
                        # stage yT to SBUF for the Pool engine; reciprocal of the
                        # denominator row; broadcast + normalize on Pool.
                        yts = ytsp.tile([HD, 256], F32, name="yts", tag="yts")
                        nc.scalar.copy(yts[:], ytp[0:HD, :])
                        rr = rrp.tile([1, 256], F32, name="rr", tag="rr")
                        with nc.allow_low_precision(reason="softmax denom reciprocal in fp32r"):
                            nc.vector.reciprocal(_r(rr[:]), ytp[HD : HD + 1, :])
                        sp = ytp_pool.tile([HD + 1, 256], F32, name="sp", tag="yt")
                        nc.tensor.matmul(
                            sp[0:64, 0:256], _r(ones64[:]), _r(rr[:]), start=True, stop=True
                        )
                        ssb = rbp.tile([HD, 256], F32, name="ssb", tag="ssb")
                        nc.scalar.copy(ssb[:], sp[0:64, 0:256])
                        nc.gpsimd.tensor_tensor(
                            _r(yn_sb[m][hh * 64 : hh * 64 + 64, qb * 256 : qb * 256 + 256]),
                            yts[:],
                            ssb[:],
                            mybir.AluOpType.mult,
                        )

            # ---- output projection (token-major out) ----
            for tt in range(4):
                ob = outp.tile([128, C], F32, name="ob", tag="ob")
                for n2 in range(2):
                    op = pj.tile([128, 512], F32, name="op", tag="pj")
                    for kc in range(KC):
                        nc.tensor.matmul(
                            op[:],
                            _r(yn_sb[kc][:, tt * 128 : (tt + 1) * 128]),
                            _r(wo_all[:, kc, n2 * 512 : (n2 + 1) * 512]),
                            start=(kc == 0),
                            stop=(kc == KC - 1),
                        )
                    nc.scalar.copy(ob[:, n2 * 512 : (n2 + 1) * 512], op[:])
                nc.sync.dma_start(out=out[tt * 128 : (tt + 1) * 128, :], in_=ob[:])

    _split_multi_waits(nc)
    return nc


_NC = None


def _get_nc():
    global _NC
    if _NC is None:
        _NC = _build_program()
    return _NC


def _host_prep(x, Wq, bq, Wk, bk, Wv, bv, Wo, bo):
    x = np.ascontiguousarray(np.asarray(x, dtype=np.float32))
    Wq = np.asarray(Wq, np.float32)
    bq = np.asarray(bq, np.float32)
    Wk = np.asarray(Wk, np.float32)
    bk = np.asarray(bk, np.float32)
    Wv = np.asarray(Wv, np.float32)
    bv = np.asarray(bv, np.float32)
    Wo = np.asarray(Wo, np.float32)
    bo = np.asarray(bo, np.float32)

    scale = np.float32(1.0 / np.sqrt(HD))
    # (m, kc)-tiled, pre-scaled Wq: wq_t[m, kc] = Wq[kc-tile, m-tile] * scale
    wq_t = np.ascontiguousarray(
        (Wq * scale).reshape(KC, 128, KC, 128).transpose(2, 1, 0, 3)
    )
    bq_h = np.ascontiguousarray((bq * scale).reshape(C, 1))
    wk_h = np.ascontiguousarray(Wk.reshape(KC, 128, KV).transpose(1, 0, 2))
    wv_h = np.ascontiguousarray(Wv.reshape(KC, 128, KV).transpose(1, 0, 2))
    wo_h = np.ascontiguousarray(Wo.reshape(KC, 128, C).transpose(1, 0, 2))
    bk_h = np.ascontiguousarray(bk.reshape(KV, 1))

    # band masks, block-packed: maskp[p, kt, qb*256+qi] for kj_l = kt*128+p
    kj = np.arange(2 * WINDOW)[:, None]  # 512 local k indices within a q-block
    qi = np.arange(WINDOW)[None, :]  # 256 local q indices within a q-block
    band = (qi <= kj) & (kj <= qi + WINDOW)  # same for every block
    masks = {}
    for c in range(4):
        mk = np.empty((128, 4, TQ), np.float32)
        for qb in range(2):
            valid = band.copy()
            if c == 0 and qb == 0:
                valid &= kj >= WINDOW  # global j >= 0 at the sequence start
            mkb = np.where(valid, np.float32(0.0), np.float32(NEG))
            mk[:, :, qb * 256 : (qb + 1) * 256] = (
                mkb.reshape(4, 128, WINDOW).transpose(1, 0, 2)
            )
        masks[c] = mk

    in_maps = []
    for core in range(NCORES):
        b, c = core // 4, core % 4
        t0 = c * TQ - WINDOW
        xe = np.zeros((TE, C), np.float32)
        lo = max(t0, 0)
        xe[lo - t0 : TE, :] = x[b, lo : t0 + TE, :]
        in_maps.append(
            {
                "xt": np.ascontiguousarray(xe.T.reshape(KC, 128, TE).transpose(1, 0, 2)),
                "wq": wq_t,
                "wk": wk_h,
                "wv": wv_h,
                "wo": wo_h,
                "bq": bq_h,
                "bk": bk_h,
                "maskp": masks[c],
            }
        )

    # exact linear bias correction applied host-side:
    # y = att@(v+bv) = att@v + bv (softmax rows sum to 1), so
    # out += bv_rep @ Wo + bo
    bv_rep = np.concatenate([bv[(h // NG) * HD : (h // NG + 1) * HD] for h in range(NH)])
    corr = bv_rep.astype(np.float64) @ Wo.astype(np.float64) + bo.astype(np.float64)
    return in_maps, corr.astype(np.float32)


LAST_RESULTS = None


def kernel(x, Wq, bq, Wk, bk, Wv, bv, Wo, bo):
    global LAST_RESULTS
    in_maps, corr = _host_prep(x, Wq, bq, Wk, bk, Wv, bv, Wo, bo)
    nc = _get_nc()
    res = run_bass_kernel_spmd(nc, in_maps, core_ids=list(range(NCORES)))
    LAST_RESULTS = res
    out = np.empty((B, T, C), np.float32)
    for core in range(NCORES):
        b, c = core // 4, core % 4
        out[b, c * TQ : (c + 1) * TQ, :] = res.results[core]["out"]
    out += corr[None, None, :]
    return out
